# revision 11
# baseline (speedup 1.0000x reference)
"""NemotronH MoE kernel for 8 Trainium2 NeuronCores.

Sharding: expert-parallel. Each of the 8 cores gets 4 of the 32 routed
experts plus a 1/8 tensor-parallel slice (along the intermediate dim S)
of the shared expert. The gate/router is replicated and computed on every
core in fp32. Each core produces a partial [T, H] output (its experts'
combine-weighted contributions + its shared-expert slice); the host sums
the 8 partials.

Device algorithm (per core, all matmuls on the PE array):
  - gate logits [T,E] in fp32, sigmoid, grouped top-k computed exactly
    with DVE Max8/threshold ops (bit-identical expert selection to the
    jax reference), combine weights renormalized and pre-scaled by 2.5.
  - combine column for each local expert is transposed (PE transpose)
    and broadcast across partitions with a tiny selector matmul.
  - per "expert" (4 routed + 1 shared slice, identical shapes H->1024->H):
      up:   h^T[I,T]   = w_up^T  @ x^T      (bf16, psum f32)
      act:  r2 = relu(h)*h (DVE, f32), hsc = bf16(r2 * combine_bcast)
      down: out[T,H]  += hsc^T.T @ w_down   (bf16, accumulated in f32)
"""

import os
import sys

import numpy as np
import ml_dtypes

for _p in ("/opt/trn_rl_repo",):
    if _p not in sys.path:
        sys.path.insert(0, _p)

import concourse.bass as bass
import concourse.mybir as mybir
import concourse.tile as tile
from concourse import bacc
from concourse.bass import ts
from concourse.masks import make_identity

BF16 = mybir.dt.bfloat16
F32 = mybir.dt.float32

T = 256          # tokens
H = 2048         # hidden
E = 32           # routed experts (global)
I = 1024         # routed expert intermediate
S = 8192         # shared expert intermediate (global)
TOP_K = 8
N_GROUP = 8
GSIZE = E // N_GROUP          # 4 experts per group
TOPK_GROUP = 4
ROUTED_SCALING = 2.5
NCORES = 8
E_LOC = E // NCORES           # 4 routed experts per core
S_LOC = S // NCORES           # 1024 shared-intermediate per core
NEXP = E_LOC + 1              # + shared slice, same [H,1024]/[1024,H] shapes

KT = H // 128                 # 16 k-tiles over hidden
IT = I // 128                 # 8 i-tiles over intermediate
TT = T // 128                 # 2 token tiles
HC = H // 512                 # 4 output column chunks


def _build_kernel():
    nc = bacc.Bacc(trn_type="TRN2", target_bir_lowering=False, debug=False)

    xt32_d = nc.dram_tensor("xt32", [H, T], F32, kind="ExternalInput").ap()
    xtb_d = nc.dram_tensor("xtb", [H, T], BF16, kind="ExternalInput").ap()
    gwt_d = nc.dram_tensor("gwt", [H, E], F32, kind="ExternalInput").ap()
    bias_d = nc.dram_tensor("biasb", [128, E], F32, kind="ExternalInput").ap()
    esel_d = nc.dram_tensor("esel", [E, E_LOC * 128], F32, kind="ExternalInput").ap()
    wu_d = nc.dram_tensor("wu", [NEXP, H, I], BF16, kind="ExternalInput").ap()
    wd_d = nc.dram_tensor("wd", [NEXP, I, H], BF16, kind="ExternalInput").ap()
    out_d = nc.dram_tensor("out", [T, H], F32, kind="ExternalOutput").ap()

    with tile.TileContext(nc) as tc:
        _emit(tc, nc, xt32_d, xtb_d, gwt_d, bias_d, esel_d, wu_d, wd_d, out_d)
    nc.compile()
    return nc


def _emit(tc, nc, xt32_d, xtb_d, gwt_d, bias_d, esel_d, wu_d, wd_d, out_d):
    from contextlib import ExitStack

    ctx = ExitStack()
    with ctx:
        _env = os.environ.get
        n_ps_up = int(_env("MOE_PSUP", "3"))
        n_ps_dn = int(_env("MOE_PSDN", "3"))
        n_ps_misc = int(_env("MOE_PSMISC", "2"))
        n_wu_bufs = int(_env("MOE_WUBUFS", "8"))
        n_wd_bufs = int(_env("MOE_WDBUFS", "2"))

        consts = ctx.enter_context(tc.tile_pool(name="consts", bufs=1))
        xpool = ctx.enter_context(tc.tile_pool(name="xpool", bufs=1))
        wu_pool = ctx.enter_context(tc.tile_pool(name="wu", bufs=n_wu_bufs))
        wd_pool = ctx.enter_context(tc.tile_pool(name="wd", bufs=n_wd_bufs))
        rpool = ctx.enter_context(tc.tile_pool(name="routing", bufs=2))
        hpool = ctx.enter_context(tc.tile_pool(name="hsc", bufs=2))
        r2pool = ctx.enter_context(tc.tile_pool(name="r2", bufs=3))
        acc_pool = ctx.enter_context(tc.tile_pool(name="acc", bufs=1))
        ps_up = ctx.enter_context(
            tc.tile_pool(name="ps_up", bufs=n_ps_up, space="PSUM")
        )
        ps_dn = ctx.enter_context(
            tc.tile_pool(name="ps_dn", bufs=n_ps_dn, space="PSUM")
        )
        ps_misc = ctx.enter_context(
            tc.tile_pool(name="ps_misc", bufs=n_ps_misc, space="PSUM")
        )

        # ---- constants / activations ----
        ident = consts.tile([128, 128], F32, tag="ident")
        make_identity(nc, ident[:])

        xt32 = xpool.tile([128, KT, T], F32, tag="xt32")
        nc.sync.dma_start(xt32[:], xt32_d.rearrange("(ko p) t -> p ko t", p=128))
        xtb = xpool.tile([128, KT, T], BF16, tag="xtb")
        nc.sync.dma_start(xtb[:], xtb_d.rearrange("(ko p) t -> p ko t", p=128))
        gwt = xpool.tile([128, KT, E], F32, tag="gwt")
        nc.sync.dma_start(gwt[:], gwt_d.rearrange("(ko p) e -> p ko e", p=128))
        biasb = consts.tile([128, E], F32, tag="biasb")
        nc.sync.dma_start(biasb[:], bias_d)
        esel = consts.tile([E, E_LOC * 128], F32, tag="esel")
        nc.sync.dma_start(esel[:], esel_d)

        # ---- router: fp32 logits -> sigmoid -> grouped top-k -> combine ----
        combT = rpool.tile([E, T], F32, tag="combT")
        combs = []
        for t in range(TT):
            ps_g = ps_misc.tile([128, 512], F32, tag="misc")
            lg = ps_g[:, :E]
            for k in range(KT):
                nc.tensor.matmul(
                    lg,
                    lhsT=xt32[:, k, ts(t, 128)],
                    rhs=gwt[:, k, :],
                    start=(k == 0),
                    stop=(k == KT - 1),
                )
            scores = rpool.tile([128, E], F32, tag="scores")
            nc.scalar.activation(scores[:], lg, mybir.ActivationFunctionType.Sigmoid)
            sfc = rpool.tile([128, E], F32, tag="sfc")
            nc.vector.tensor_add(sfc[:], scores[:], biasb[:])

            # group score = max over pairwise sums = top-2 sum within each group
            sfc3 = sfc[:].rearrange("p (g j) -> p g j", j=GSIZE)
            gsum = rpool.tile([128, N_GROUP], F32, tag="gsum")
            pair = rpool.tile([128, N_GROUP], F32, tag="pair")
            first = True
            for j1 in range(GSIZE):
                for j2 in range(j1 + 1, GSIZE):
                    dst = gsum if first else pair
                    nc.vector.tensor_add(dst[:], sfc3[:, :, j1], sfc3[:, :, j2])
                    if not first:
                        nc.vector.tensor_tensor(
                            gsum[:], gsum[:], pair[:], op=mybir.AluOpType.max
                        )
                    first = False

            m8g = rpool.tile([128, 8], F32, tag="m8g")
            nc.vector.max(out=m8g[:], in_=gsum[:])
            gmask = rpool.tile([128, N_GROUP], F32, tag="gmask")
            nc.vector.tensor_scalar(
                gmask[:], gsum[:], m8g[:, TOPK_GROUP - 1 : TOPK_GROUP], None,
                op0=mybir.AluOpType.is_ge,
            )
            # masked biased scores (0 where group not selected)
            tmp = rpool.tile([128, E], F32, tag="tmpsc")
            tmp3 = tmp[:].rearrange("p (g j) -> p g j", j=GSIZE)
            nc.vector.tensor_tensor(
                tmp3,
                sfc3,
                gmask[:, :, None].to_broadcast([128, N_GROUP, GSIZE]),
                op=mybir.AluOpType.mult,
            )
            m8t = rpool.tile([128, 8], F32, tag="m8t")
            nc.vector.max(out=m8t[:], in_=tmp[:])
            sel = rpool.tile([128, E], F32, tag="sel")
            nc.vector.tensor_scalar(
                sel[:], tmp[:], m8t[:, TOP_K - 1 : TOP_K], None,
                op0=mybir.AluOpType.is_ge,
            )
            wraw = rpool.tile([128, E], F32, tag="wraw")
            nc.vector.tensor_mul(wraw[:], scores[:], sel[:])
            denom = rpool.tile([128, 1], F32, tag="denom")
            nc.vector.reduce_sum(denom[:], wraw[:], axis=mybir.AxisListType.X)
            inv = rpool.tile([128, 1], F32, tag="inv")
            nc.vector.reciprocal(inv[:], denom[:])
            comb = rpool.tile([128, E], F32, tag="comb")
            nc.vector.tensor_scalar(
                comb[:], wraw[:], inv[:], float(ROUTED_SCALING),
                op0=mybir.AluOpType.mult, op1=mybir.AluOpType.mult,
            )
            # combT[:, t*128:(t+1)*128] = comb.T  (PE transpose emitted here,
            # but it's scheduled after the first expert's up-matmuls keep PE
            # busy; comb stays live in its pool slot)
            combs.append(comb)

        # ---- experts: shared slice first (needs no combine weights -> its
        # up-matmuls hide the routing DVE chain), then the 4 routed ----
        acc = [
            acc_pool.tile([128, H], F32, tag=f"acc{t}", name=f"acc{t}")
            for t in range(TT)
        ]
        cbc = []

        WU_CH = 4   # wu k-tiles per DMA chunk
        for idx, e in enumerate([E_LOC] + list(range(E_LOC))):
            wu_sb = []
            for ch in range(KT // WU_CH):
                w = wu_pool.tile([128, WU_CH, I], BF16, tag="wu", name=f"wu{e}_{ch}")
                nc.sync.dma_start(
                    w[:],
                    wu_d[e, ch * WU_CH * 128 : (ch + 1) * WU_CH * 128, :].rearrange(
                        "(ko p) i -> p ko i", p=128
                    ),
                )
                wu_sb.append(w)
            wd_sb = wd_pool.tile([128, IT, H], BF16, tag="wd")
            nc.sync.dma_start(
                wd_sb[:], wd_d[e].rearrange("(io p) h -> p io h", p=128)
            )

            hsc = []
            for i in range(IT):
                ps_h = ps_up.tile([128, T], F32, tag="ps_h")
                for k in range(KT):
                    nc.tensor.matmul(
                        ps_h[:],
                        lhsT=wu_sb[k // WU_CH][:, k % WU_CH, ts(i, 128)],
                        rhs=xtb[:, k, :],
                        start=(k == 0),
                        stop=(k == KT - 1),
                    )
                h = hpool.tile([128, T], BF16, tag=f"hsc{i}")
                if e < E_LOC:
                    # h = relu(hp) * (hp * c) = c * relu2(hp); one psum read per op
                    rc = r2pool.tile([128, T], F32, tag="rc")
                    nc.vector.tensor_mul(rc[:], ps_h[:], cbc[e][:])
                    nc.vector.scalar_tensor_tensor(
                        h[:], ps_h[:], 0.0, rc[:],
                        op0=mybir.AluOpType.max, op1=mybir.AluOpType.mult,
                    )
                else:
                    r = r2pool.tile([128, T], F32, tag="rr")
                    nc.scalar.activation(
                        r[:], ps_h[:], mybir.ActivationFunctionType.Relu
                    )
                    nc.vector.tensor_mul(h[:], r[:], r[:])
                hsc.append(h)

            if idx == 0:
                # routing epilogue: transpose combine, broadcast each local
                # expert's column across partitions (PE is warm from shared-up)
                for t in range(TT):
                    ps_tr = ps_misc.tile([128, 512], F32, tag="misc")
                    nc.tensor.transpose(ps_tr[:E, :128], combs[t][:], ident[:])
                    nc.vector.tensor_copy(combT[:, ts(t, 128)], ps_tr[:E, :128])
                for le in range(E_LOC):
                    ps_bc = ps_misc.tile([128, 512], F32, tag="misc")
                    nc.tensor.matmul(
                        ps_bc[:, :T],
                        lhsT=esel[:, ts(le, 128)],
                        rhs=combT[:],
                        start=True,
                        stop=True,
                    )
                    cb = rpool.tile([128, T], F32, tag=f"cbc{le}", name=f"cbc{le}")
                    nc.vector.tensor_copy(cb[:], ps_bc[:, :T])
                    cbc.append(cb)

            for t in range(TT):
                for c in range(HC):
                    ps_d = ps_dn.tile([128, 512], F32, tag="ps_d")
                    for i in range(IT):
                        nc.tensor.matmul(
                            ps_d[:],
                            lhsT=hsc[i][:, ts(t, 128)],
                            rhs=wd_sb[:, i, ts(c, 512)],
                            start=(i == 0),
                            stop=(i == IT - 1),
                        )
                    a = acc[t][:, ts(c, 512)]
                    if idx == 0:
                        nc.vector.tensor_copy(a, ps_d[:])
                    else:
                        nc.vector.tensor_add(a, ps_d[:], a)
                    if idx == NEXP - 1:
                        nc.sync.dma_start(
                            out_d[ts(t, 128), ts(c, 512)], a
                        )


def _prep_inputs(hidden_states, gate_w, correction_bias, w_up, w_down, ws_up, ws_down):
    """Host-side sharding/layout prep. Returns per-core input maps."""
    bf = ml_dtypes.bfloat16
    x = np.ascontiguousarray(hidden_states.astype(np.float32))
    xt = np.ascontiguousarray(x.T)                        # [H, T] f32
    xtb = xt.astype(bf)
    gwt = np.ascontiguousarray(gate_w.astype(np.float32).T)   # [H, E]
    biasb = np.broadcast_to(
        correction_bias.astype(np.float32)[None, :], (128, E)
    ).copy()

    in_maps = []
    for c in range(NCORES):
        esel = np.zeros((E, E_LOC * 128), np.float32)
        for e in range(E_LOC):
            esel[c * E_LOC + e, e * 128 : (e + 1) * 128] = 1.0
        wu = np.empty((NEXP, H, I), bf)
        wd = np.empty((NEXP, I, H), bf)
        wu[:E_LOC] = w_up[c * E_LOC : (c + 1) * E_LOC].astype(bf)
        wd[:E_LOC] = w_down[c * E_LOC : (c + 1) * E_LOC].astype(bf)
        wu[E_LOC] = ws_up[:, c * S_LOC : (c + 1) * S_LOC].astype(bf)
        wd[E_LOC] = ws_down[c * S_LOC : (c + 1) * S_LOC, :].astype(bf)
        in_maps.append(
            {
                "xt32": xt,
                "xtb": xtb,
                "gwt": gwt,
                "biasb": biasb,
                "esel": esel,
                "wu": wu,
                "wd": wd,
            }
        )
    return in_maps


_CACHED = {}


def _get_nc():
    if "nc" not in _CACHED:
        _CACHED["nc"] = _build_kernel()
    return _CACHED["nc"]


def kernel(hidden_states, gate_w, correction_bias, w_up, w_down, ws_up, ws_down):
    from concourse.bass_utils import run_bass_kernel_spmd

    nc = _get_nc()
    in_maps = _prep_inputs(
        hidden_states, gate_w, correction_bias, w_up, w_down, ws_up, ws_down
    )
    res = run_bass_kernel_spmd(nc, in_maps, list(range(NCORES)))
    out = np.zeros((T, H), np.float32)
    for r in res.results:
        out += r["out"]
    return out


# revision 26
# speedup vs baseline: 16.0925x; 16.0925x over previous
"""NemotronH MoE kernel for 8 Trainium2 NeuronCores.

Sharding: expert-parallel. Each of the 8 cores gets 4 of the 32 routed
experts plus a 1/8 tensor-parallel slice (along the intermediate dim S)
of the shared expert. The gate/router is replicated and computed on every
core in fp32. Each core produces a partial [T, H] output (its experts'
combine-weighted contributions + its shared-expert slice); the host sums
the 8 partials.

Device algorithm (per core, all matmuls on the PE array):
  - gate logits [T,E] in fp32, sigmoid, grouped top-k computed exactly
    with DVE Max8/threshold ops (bit-identical expert selection to the
    jax reference), combine weights renormalized and pre-scaled by 2.5.
  - combine column for each local expert is transposed (PE transpose)
    and broadcast across partitions with a tiny selector matmul.
  - per "expert" (4 routed + 1 shared slice, identical shapes H->1024->H):
      up:   h^T[I,T]   = w_up^T  @ x^T      (bf16, psum f32)
      act:  r2 = relu(h)*h (DVE, f32), hsc = bf16(r2 * combine_bcast)
      down: out[T,H]  += hsc^T.T @ w_down   (bf16, accumulated in f32)
"""

import os
import sys

import numpy as np
import ml_dtypes

for _p in ("/opt/trn_rl_repo",):
    if _p not in sys.path:
        sys.path.insert(0, _p)

import concourse.bass as bass
import concourse.mybir as mybir
import concourse.tile as tile
from concourse import bacc
from concourse.bass import ts
from concourse.masks import make_identity

BF16 = mybir.dt.bfloat16
F32 = mybir.dt.float32

T = 256          # tokens
H = 2048         # hidden
E = 32           # routed experts (global)
I = 1024         # routed expert intermediate
S = 8192         # shared expert intermediate (global)
TOP_K = 8
N_GROUP = 8
GSIZE = E // N_GROUP          # 4 experts per group
TOPK_GROUP = 4
ROUTED_SCALING = 2.5
NCORES = 8
E_LOC = E // NCORES           # 4 routed experts per core
S_LOC = S // NCORES           # 1024 shared-intermediate per core
NEXP = E_LOC + 1              # + shared slice, same [H,1024]/[1024,H] shapes

KT = H // 128                 # 16 k-tiles over hidden
IT = I // 128                 # 8 i-tiles over intermediate
TT = T // 128                 # 2 token tiles
HC = H // 512                 # 4 output column chunks


def _build_kernel():
    nc = bacc.Bacc(trn_type="TRN2", target_bir_lowering=False, debug=False)

    xt32_d = nc.dram_tensor("xt32", [H, T], F32, kind="ExternalInput").ap()
    xtb_d = nc.dram_tensor("xtb", [H, T], BF16, kind="ExternalInput").ap()
    gwt_d = nc.dram_tensor("gwt", [H, E], F32, kind="ExternalInput").ap()
    bias_d = nc.dram_tensor("biasb", [128, E], F32, kind="ExternalInput").ap()
    esel_d = nc.dram_tensor("esel", [E, E_LOC * 128], F32, kind="ExternalInput").ap()
    wu_d = nc.dram_tensor("wu", [NEXP, H, I], BF16, kind="ExternalInput").ap()
    wd_d = nc.dram_tensor("wd", [NEXP, I, H], BF16, kind="ExternalInput").ap()
    out_d = nc.dram_tensor("out", [T, H], F32, kind="ExternalOutput").ap()

    with tile.TileContext(nc) as tc:
        _emit(tc, nc, xt32_d, xtb_d, gwt_d, bias_d, esel_d, wu_d, wd_d, out_d)
    nc.compile()
    return nc


def _emit(tc, nc, xt32_d, xtb_d, gwt_d, bias_d, esel_d, wu_d, wd_d, out_d):
    from contextlib import ExitStack

    ctx = ExitStack()
    with ctx:
        _env = os.environ.get
        n_ps_up = int(_env("MOE_PSUP", "5"))
        n_ps_dn = int(_env("MOE_PSDN", "2"))
        n_ps_misc = int(_env("MOE_PSMISC", "1"))
        wu_ch = int(_env("MOE_WUCH", "4"))
        n_wu_bufs = int(_env("MOE_WUBUFS", str(2 * (KT // wu_ch))))
        n_wd_bufs = int(_env("MOE_WDBUFS", "4"))

        consts = ctx.enter_context(tc.tile_pool(name="consts", bufs=1))
        xpool = ctx.enter_context(tc.tile_pool(name="xpool", bufs=1))
        wu_pool = ctx.enter_context(tc.tile_pool(name="wu", bufs=n_wu_bufs))
        wd_pool = ctx.enter_context(tc.tile_pool(name="wd", bufs=n_wd_bufs))
        rpool = ctx.enter_context(tc.tile_pool(name="routing", bufs=2))
        hpool = ctx.enter_context(tc.tile_pool(name="hsc", bufs=2))
        r2pool = ctx.enter_context(tc.tile_pool(name="r2", bufs=3))
        acc_pool = ctx.enter_context(tc.tile_pool(name="acc", bufs=1))
        ps_up = ctx.enter_context(
            tc.tile_pool(name="ps_up", bufs=n_ps_up, space="PSUM")
        )
        ps_dn = ctx.enter_context(
            tc.tile_pool(name="ps_dn", bufs=n_ps_dn, space="PSUM")
        )
        ps_misc = ctx.enter_context(
            tc.tile_pool(name="ps_misc", bufs=n_ps_misc, space="PSUM")
        )

        # ---- constants / activations ----
        ident = consts.tile([128, 128], F32, tag="ident")
        make_identity(nc, ident[:])

        # x^T in bf16, chunked so the first up-matmuls start after ~0.5MB of
        # DMA; chunk DMAs are interleaved with the first expert's wu chunks
        # below to match the k-outer consumption order
        XCH = 4
        xtb_sb = []

        def emit_xtb_dma(ch):
            xt = xpool.tile([128, XCH, T], BF16, tag=f"xtb{ch}", name=f"xtb{ch}")
            nc.sync.dma_start(
                xt[:],
                xtb_d[ch * XCH * 128 : (ch + 1) * XCH * 128, :].rearrange(
                    "(ko p) t -> p ko t", p=128
                ),
            )
            xtb_sb.append(xt)

        def xtb(k):
            return xtb_sb[k // XCH][:, k % XCH, :]

        # shared-expert weights stream next (see expert loop below); the fp32
        # gate inputs follow them in DMA order since the router only runs
        # after the shared expert's matmuls
        def emit_gate_inputs():
            gwt = xpool.tile([128, KT, E], F32, tag="gwt")
            nc.sync.dma_start(gwt[:], gwt_d.rearrange("(ko p) e -> p ko e", p=128))
            xt32 = xpool.tile([128, KT, T], F32, tag="xt32")
            nc.sync.dma_start(
                xt32[:], xt32_d.rearrange("(ko p) t -> p ko t", p=128)
            )
            biasb = consts.tile([128, E], F32, tag="biasb")
            nc.sync.dma_start(biasb[:], bias_d)
            esel = consts.tile([E, E_LOC * 128], F32, tag="esel")
            nc.sync.dma_start(esel[:], esel_d)
            return gwt, xt32, biasb, esel

        # ---- router: fp32 logits -> sigmoid -> grouped top-k -> combine ----
        combT = rpool.tile([E, T], F32, tag="combT")
        combs = []

        def emit_routing(t, gwt, xt32, biasb):
            ps_g = ps_misc.tile([128, 512], F32, tag="misc")
            lg = ps_g[:, :E]
            for k in range(KT):
                nc.tensor.matmul(
                    lg,
                    lhsT=xt32[:, k, ts(t, 128)],
                    rhs=gwt[:, k, :],
                    start=(k == 0),
                    stop=(k == KT - 1),
                )
            scores = rpool.tile([128, E], F32, tag="scores")
            nc.scalar.activation(scores[:], lg, mybir.ActivationFunctionType.Sigmoid)
            sfc = rpool.tile([128, E], F32, tag="sfc")
            nc.vector.tensor_add(sfc[:], scores[:], biasb[:])

            # group score = max over pairwise sums = top-2 sum within each group
            sfc3 = sfc[:].rearrange("p (g j) -> p g j", j=GSIZE)
            gsum = rpool.tile([128, N_GROUP], F32, tag="gsum")
            pair = rpool.tile([128, N_GROUP], F32, tag="pair")
            first = True
            for j1 in range(GSIZE):
                for j2 in range(j1 + 1, GSIZE):
                    dst = gsum if first else pair
                    nc.vector.tensor_add(dst[:], sfc3[:, :, j1], sfc3[:, :, j2])
                    if not first:
                        nc.vector.tensor_tensor(
                            gsum[:], gsum[:], pair[:], op=mybir.AluOpType.max
                        )
                    first = False

            m8g = rpool.tile([128, 8], F32, tag="m8g")
            nc.vector.max(out=m8g[:], in_=gsum[:])
            gmask = rpool.tile([128, N_GROUP], F32, tag="gmask")
            nc.vector.tensor_scalar(
                gmask[:], gsum[:], m8g[:, TOPK_GROUP - 1 : TOPK_GROUP], None,
                op0=mybir.AluOpType.is_ge,
            )
            # masked biased scores (0 where group not selected)
            tmp = rpool.tile([128, E], F32, tag="tmpsc")
            tmp3 = tmp[:].rearrange("p (g j) -> p g j", j=GSIZE)
            nc.vector.tensor_tensor(
                tmp3,
                sfc3,
                gmask[:, :, None].to_broadcast([128, N_GROUP, GSIZE]),
                op=mybir.AluOpType.mult,
            )
            m8t = rpool.tile([128, 8], F32, tag="m8t")
            nc.vector.max(out=m8t[:], in_=tmp[:])
            sel = rpool.tile([128, E], F32, tag="sel")
            nc.vector.tensor_scalar(
                sel[:], tmp[:], m8t[:, TOP_K - 1 : TOP_K], None,
                op0=mybir.AluOpType.is_ge,
            )
            wraw = rpool.tile([128, E], F32, tag="wraw")
            nc.vector.tensor_mul(wraw[:], scores[:], sel[:])
            denom = rpool.tile([128, 1], F32, tag="denom")
            nc.vector.reduce_sum(denom[:], wraw[:], axis=mybir.AxisListType.X)
            inv = rpool.tile([128, 1], F32, tag="inv")
            nc.vector.reciprocal(inv[:], denom[:])
            comb = rpool.tile([128, E], F32, tag="comb")
            nc.vector.tensor_scalar(
                comb[:], wraw[:], inv[:], float(ROUTED_SCALING),
                op0=mybir.AluOpType.mult, op1=mybir.AluOpType.mult,
            )
            # combT[:, t*128:(t+1)*128] = comb.T  (PE transpose emitted here,
            # but it's scheduled after the first expert's up-matmuls keep PE
            # busy; comb stays live in its pool slot)
            combs.append(comb)

        # ---- experts: shared slice first (needs no combine weights -> its
        # up-matmuls hide the routing DVE chain), then the 4 routed ----
        acc = [
            acc_pool.tile([128, H], F32, tag=f"acc{t}", name=f"acc{t}")
            for t in range(TT)
        ]
        cbc = []

        WU_CH = wu_ch   # wu k-tiles per DMA chunk

        def emit_wu_dma(e, wu_sb=None, interleave_xtb=False):
            if wu_sb is None:
                wu_sb = []
            for ch in range(KT // WU_CH):
                if interleave_xtb and ch < KT // XCH:
                    emit_xtb_dma(ch)
                w = wu_pool.tile([128, WU_CH, I], BF16, tag="wu", name=f"wu{e}_{ch}")
                nc.sync.dma_start(
                    w[:],
                    wu_d[e, ch * WU_CH * 128 : (ch + 1) * WU_CH * 128, :].rearrange(
                        "(ko p) i -> p ko i", p=128
                    ),
                )
                wu_sb.append(w)
            return wu_sb

        WD_CH = 4   # wd i-tiles per DMA chunk

        def emit_wd_dma(e, half):
            w = wd_pool.tile([128, WD_CH, H], BF16, tag="wd", name=f"wd{e}_{half}")
            nc.sync.dma_start(
                w[:],
                wd_d[e, half * WD_CH * 128 : (half + 1) * WD_CH * 128, :].rearrange(
                    "(io p) h -> p io h", p=128
                ),
            )
            return w

        UPH = IT // 2   # i-tiles per up half

        def emit_up_half(e, wu_sb, ih, hsc):
            # k-outer over a half of the i-tiles: the first matmuls only need
            # the first wu chunk, so DMA consumption is progressive
            pss = [
                ps_up.tile([128, T], F32, tag="ps_h", name=f"ps{e}_{ih}_{i}")
                for i in range(UPH)
            ]
            for k in range(KT):
                for i in range(UPH):
                    nc.tensor.matmul(
                        pss[i][:],
                        lhsT=wu_sb[k // WU_CH][:, k % WU_CH, ts(ih * UPH + i, 128)],
                        rhs=xtb(k),
                        start=(k == 0),
                        stop=(k == KT - 1),
                    )
            for i in range(UPH):
                ii = ih * UPH + i
                h = hpool.tile([128, T], BF16, tag=f"hsc{ii}")
                if e < E_LOC:
                    # h = relu(hp) * (hp * c) = c * relu2(hp); one psum read/op
                    rc = r2pool.tile([128, T], F32, tag="rc")
                    nc.vector.tensor_mul(rc[:], pss[i][:], cbc[e][:])
                    nc.vector.scalar_tensor_tensor(
                        h[:], pss[i][:], 0.0, rc[:],
                        op0=mybir.AluOpType.max, op1=mybir.AluOpType.mult,
                    )
                else:
                    r = r2pool.tile([128, T], F32, tag="rr")
                    nc.scalar.activation(
                        r[:], pss[i][:], mybir.ActivationFunctionType.Relu
                    )
                    nc.vector.tensor_mul(h[:], r[:], r[:])
                hsc.append(h)

        def emit_routing_epilogue():
            # transpose combine, broadcast each local expert's column across
            # partitions (PE is warm from shared-up)
            for t in range(TT):
                ps_tr = ps_misc.tile([128, 512], F32, tag="misc")
                nc.tensor.transpose(ps_tr[:E, :128], combs[t][:], ident[:])
                nc.vector.tensor_copy(combT[:, ts(t, 128)], ps_tr[:E, :128])
            for le in range(E_LOC):
                ps_bc = ps_misc.tile([128, 512], F32, tag="misc")
                nc.tensor.matmul(
                    ps_bc[:, :T],
                    lhsT=esel[:, ts(le, 128)],
                    rhs=combT[:],
                    start=True,
                    stop=True,
                )
                cb = rpool.tile([128, T], F32, tag=f"cbc{le}", name=f"cbc{le}")
                nc.vector.tensor_copy(cb[:], ps_bc[:, :T])
                cbc.append(cb)

        def emit_down_half(idx, half, hsc, wd_half, init, final):
            # contributions of i-tiles [half*WD_CH, (half+1)*WD_CH) to all
            # (t,c) output chunks; accumulated into acc via DVE
            for t in range(TT):
                for c in range(HC):
                    ps_d = ps_dn.tile([128, 512], F32, tag="ps_d")
                    for i in range(WD_CH):
                        nc.tensor.matmul(
                            ps_d[:],
                            lhsT=hsc[half * WD_CH + i][:, ts(t, 128)],
                            rhs=wd_half[:, i, ts(c, 512)],
                            start=(i == 0),
                            stop=(i == WD_CH - 1),
                        )
                    a = acc[t][:, ts(c, 512)]
                    if init:
                        nc.vector.tensor_copy(a, ps_d[:])
                    else:
                        nc.vector.tensor_add(a, ps_d[:], a)
                    if final:
                        nc.sync.dma_start(out_d[ts(t, 128), ts(c, 512)], a)

        def emit_down_full(idx, hsc, wd_halves):
            for t in range(TT):
                for c in range(HC):
                    ps_d = ps_dn.tile([128, 512], F32, tag="ps_d")
                    for i in range(IT):
                        nc.tensor.matmul(
                            ps_d[:],
                            lhsT=hsc[i][:, ts(t, 128)],
                            rhs=wd_halves[i // WD_CH][:, i % WD_CH, ts(c, 512)],
                            start=(i == 0),
                            stop=(i == IT - 1),
                        )
                    a = acc[t][:, ts(c, 512)]
                    nc.vector.tensor_add(a, ps_d[:], a)
                    if idx == NEXP - 1:
                        nc.sync.dma_start(out_d[ts(t, 128), ts(c, 512)], a)

        # ---- phase 1: shared expert, gate+routing interleaved; the two
        # down halves straddle the gate so the fp32 gate inputs and second
        # wd chunk can arrive later ----
        sh = E_LOC
        wu_sb = emit_wu_dma(sh, interleave_xtb=True)
        wd0 = emit_wd_dma(sh, 0)
        gwt, xt32, biasb, esel = emit_gate_inputs()
        wd1 = emit_wd_dma(sh, 1)

        hsc = []
        emit_up_half(sh, wu_sb, 0, hsc)
        emit_up_half(sh, wu_sb, 1, hsc)
        emit_down_half(0, 0, hsc, wd0, True, False)
        for t in range(TT):
            emit_routing(t, gwt, xt32, biasb)
        emit_down_half(0, 1, hsc, wd1, False, False)
        emit_routing_epilogue()

        # ---- phase 2: routed experts ----
        for idx, e in enumerate(range(E_LOC), start=1):
            wu_sb = emit_wu_dma(e)
            wd0 = emit_wd_dma(e, 0)
            wd1 = emit_wd_dma(e, 1)
            hsc = []
            emit_up_half(e, wu_sb, 0, hsc)
            emit_up_half(e, wu_sb, 1, hsc)
            emit_down_full(idx, hsc, [wd0, wd1])


def _prep_inputs(hidden_states, gate_w, correction_bias, w_up, w_down, ws_up, ws_down):
    """Host-side sharding/layout prep. Returns per-core input maps."""
    bf = ml_dtypes.bfloat16
    hidden_states = np.asarray(hidden_states)
    gate_w = np.asarray(gate_w)
    correction_bias = np.asarray(correction_bias)
    w_up = np.asarray(w_up)
    w_down = np.asarray(w_down)
    ws_up = np.asarray(ws_up)
    ws_down = np.asarray(ws_down)
    x = np.ascontiguousarray(hidden_states.astype(np.float32))
    xt = np.ascontiguousarray(x.T)                        # [H, T] f32
    xtb = xt.astype(bf)
    gwt = np.ascontiguousarray(gate_w.astype(np.float32).T)   # [H, E]
    biasb = np.broadcast_to(
        correction_bias.astype(np.float32)[None, :], (128, E)
    ).copy()

    in_maps = []
    for c in range(NCORES):
        esel = np.zeros((E, E_LOC * 128), np.float32)
        for e in range(E_LOC):
            esel[c * E_LOC + e, e * 128 : (e + 1) * 128] = 1.0
        wu = np.empty((NEXP, H, I), bf)
        wd = np.empty((NEXP, I, H), bf)
        wu[:E_LOC] = w_up[c * E_LOC : (c + 1) * E_LOC].astype(bf)
        wd[:E_LOC] = w_down[c * E_LOC : (c + 1) * E_LOC].astype(bf)
        wu[E_LOC] = ws_up[:, c * S_LOC : (c + 1) * S_LOC].astype(bf)
        wd[E_LOC] = ws_down[c * S_LOC : (c + 1) * S_LOC, :].astype(bf)
        in_maps.append(
            {
                "xt32": xt,
                "xtb": xtb,
                "gwt": gwt,
                "biasb": biasb,
                "esel": esel,
                "wu": wu,
                "wd": wd,
            }
        )
    return in_maps


_CACHED = {}


def _get_nc():
    if "nc" not in _CACHED:
        _CACHED["nc"] = _build_kernel()
    return _CACHED["nc"]


def kernel(hidden_states, gate_w, correction_bias, w_up, w_down, ws_up, ws_down):
    from concourse.bass_utils import run_bass_kernel_spmd

    nc = _get_nc()
    in_maps = _prep_inputs(
        hidden_states, gate_w, correction_bias, w_up, w_down, ws_up, ws_down
    )
    res = run_bass_kernel_spmd(nc, in_maps, list(range(NCORES)))
    out = np.zeros((T, H), np.float32)
    for r in res.results:
        out += r["out"]
    return out
